# revision 20
# baseline (speedup 1.0000x reference)
"""MoE gate kernel for Trainium2 (8 NeuronCores, SPMD data-parallel over tokens).

reference:
    scores = sigmoid(x @ W.T)            # [T, E] fp32
    biased = scores + bias
    inds   = top_k(-biased, 8).indices   # 8 smallest biased, order ascending biased
    sel    = scores[inds] / sum * 2.5

Device strategy (per core, 2048 tokens):
  - x and W.T are split on the host into fp16 hi + fp16 lo residual pairs
    (22-bit mantissa coverage -> logits match the fp32 reference to ~1e-6;
    lo residuals may be fp16 subnormals, which the PE multiplies exactly).
    x is additionally pre-tiled on the host into the exact per-partition
    SBUF layout (h-block on partitions) so every DMA descriptor moves
    contiguous KBs instead of 128B strips.
  - logits = xh@wh + xh@wl + xl@wh  (3 fp16 matmuls, one PSUM accumulation)
    x is the stationary PE operand so PSUM comes out as [tokens, experts]:
    no transposes anywhere on device.
  - ACT sigmoid, DVE max/max_index for top-8 (matches jax tie-breaking),
    iota-equality scalar_tensor_tensor with accum_out to gather the selected
    original scores, reciprocal-normalize, scale by 2.5.
  - Output: one [128, NT*16] u32 buffer; per token 8 idx words + 8 fp32-bit
    sel words; host un-permutes.
"""

import sys

sys.path.insert(0, "/opt/trn_rl_repo")

import numpy as np

import concourse.bacc as bacc
import concourse.mybir as mybir
import concourse.tile as tile
from concourse import bass_utils

T, H, E, K = 16384, 4096, 256, 8
N_CORES = 8
TS = T // N_CORES          # tokens per core
TCHUNK = 128               # tokens per PE tile (PSUM partition dim)
NT = TS // TCHUNK          # token tiles per core
F = H // 128               # h-slices per partition block
ROUTED_SCALING = 2.5

f32 = mybir.dt.float32
f16 = mybir.dt.float16
u32 = mybir.dt.uint32
Alu = mybir.AluOpType
Act = mybir.ActivationFunctionType


def build_nc(nt=NT):
    """Build the SPMD Bass program for one core handling nt*TCHUNK tokens."""
    nc = bacc.Bacc("TRN2", target_bir_lowering=False, debug=False,
                   num_devices=N_CORES)

    # x pre-tiled on host: [it, p, f*TCHUNK + t] = x[it*TCHUNK + t, 32p + f]
    xth_d = nc.dram_tensor("xth", [nt, 128, F * TCHUNK], f16,
                           kind="ExternalInput")
    xtl_d = nc.dram_tensor("xtl", [nt, 128, F * TCHUNK], f16,
                           kind="ExternalInput")
    wth_d = nc.dram_tensor("wth", [H, E], f16, kind="ExternalInput")
    wtl_d = nc.dram_tensor("wtl", [H, E], f16, kind="ExternalInput")
    nbias_d = nc.dram_tensor("nbias", [128, E], f32, kind="ExternalInput")
    iota_d = nc.dram_tensor("iota", [128, E], f32, kind="ExternalInput")
    out_d = nc.dram_tensor("out", [128, nt * 2 * K], u32, kind="ExternalOutput")

    with tile.TileContext(nc) as tc:
        with (
            tc.tile_pool(name="const", bufs=1) as cpool,
            tc.tile_pool(name="xp", bufs=4) as xpool,
            tc.tile_pool(name="sc", bufs=4) as spool,
            tc.tile_pool(name="sm", bufs=4) as smpool,
            tc.tile_pool(name="ps", bufs=4, space="PSUM") as ppool,
        ):
            # weights + consts on the ACT HWDGE queue, x tiles on the SP
            # queue, so the startup loads run in parallel. Each weight chunk
            # is its own tile so the f=0 matmuls depend only on chunk 0.
            FC = F // 4
            wth_src = wth_d.ap().rearrange("(p f) e -> p f e", f=F)
            wtl_src = wtl_d.ap().rearrange("(p f) e -> p f e", f=F)
            wth_c, wtl_c = [], []
            for c in range(4):
                fs = slice(c * FC, (c + 1) * FC)
                th = cpool.tile([128, FC, E], f16, tag=f"wth{c}")
                nc.scalar.dma_start(th[:], wth_src[:, fs, :])
                tl = cpool.tile([128, FC, E], f16, tag=f"wtl{c}")
                nc.scalar.dma_start(tl[:], wtl_src[:, fs, :])
                wth_c.append(th)
                wtl_c.append(tl)
            nb = cpool.tile([128, E], f32, tag="nb")
            nc.scalar.dma_start(nb[:], nbias_d.ap())
            io = cpool.tile([128, E], f32, tag="io")
            nc.scalar.dma_start(io[:], iota_d.ap())
            scrv = cpool.tile([128, E], f32, tag="scrv")
            obuf = cpool.tile([128, nt * 2 * K], u32, tag="obuf")

            FH = F // 2
            for it in range(nt):
                xh_src = xth_d.ap()[it].rearrange("p (f t) -> p f t", f=F)
                xl_src = xtl_d.ap()[it].rearrange("p (f t) -> p f t", f=F)
                xh_h, xl_h = [], []
                for c in range(2):
                    fs = slice(c * FH, (c + 1) * FH)
                    th = xpool.tile([128, FH, TCHUNK], f16, tag=f"xh{c}")
                    nc.sync.dma_start(th[:], xh_src[:, fs, :])
                    tl = xpool.tile([128, FH, TCHUNK], f16, tag=f"xl{c}")
                    nc.sync.dma_start(tl[:], xl_src[:, fs, :])
                    xh_h.append(th)
                    xl_h.append(tl)

                # the lo parts are raw fp16 residuals (subnormals included —
                # the PE handles them exactly), so all three terms accumulate
                # into a single PSUM bank.
                acc = ppool.tile([128, E], f32, tag="acc")
                for f in range(F):
                    xh_f = xh_h[f // FH][:, f % FH, :]
                    xl_f = xl_h[f // FH][:, f % FH, :]
                    wth_f = wth_c[f // FC][:, f % FC, :]
                    wtl_f = wtl_c[f // FC][:, f % FC, :]
                    nc.tensor.matmul(acc[:], xh_f, wth_f,
                                     start=(f == 0), stop=False)
                    nc.tensor.matmul(acc[:], xh_f, wtl_f,
                                     start=False, stop=False)
                    nc.tensor.matmul(acc[:], xl_f, wth_f,
                                     start=False, stop=(f == F - 1))

                scores = spool.tile([128, E], f32, tag="scores")
                nc.scalar.activation(scores[:], acc[:], Act.Sigmoid)

                negb = spool.tile([128, E], f32, tag="negb")
                nc.vector.tensor_tensor(negb[:], nb[:], scores[:], Alu.subtract)
                m8 = smpool.tile([128, K], f32, tag="m8")
                idx = obuf[:, it * 2 * K: it * 2 * K + K]
                nc.vector.max(m8[:], negb[:])
                nc.vector.max_index(idx, m8[:], negb[:])
                idxf = smpool.tile([128, K], f32, tag="idxf")
                nc.vector.tensor_copy(idxf[:], idx)

                gath = smpool.tile([128, K], f32, tag="gath")
                for j in range(K):
                    nc.vector.scalar_tensor_tensor(
                        scrv[:], io[:], idxf[:, j:j + 1], scores[:],
                        Alu.is_equal, Alu.mult,
                        accum_out=gath[:, j:j + 1])

                ssum = smpool.tile([128, 1], f32, tag="ssum")
                nc.vector.tensor_reduce(ssum[:], gath[:],
                                        mybir.AxisListType.X, Alu.add)
                rec = smpool.tile([128, 1], f32, tag="rec")
                nc.vector.reciprocal(rec[:], ssum[:])

                nc.vector.tensor_scalar(
                    obuf[:, it * 2 * K + K: (it + 1) * 2 * K].bitcast(f32),
                    gath[:], rec[:], ROUTED_SCALING, Alu.mult, Alu.mult)

            nc.sync.dma_start(out_d.ap(), obuf[:])

    nc.compile()
    return nc


def host_prep(x, weight, e_score_correction_bias):
    """Split inputs into fp16 hi/lo pairs, pre-tile x, build per-core maps."""
    x = np.asarray(x, dtype=np.float32)
    w = np.asarray(weight, dtype=np.float32)
    b = np.asarray(e_score_correction_bias, dtype=np.float32)

    xh = x.astype(np.float16)
    xl = (x - xh.astype(np.float32)).astype(np.float16)

    def pretile(a):  # [TS, H] -> [NT, 128, F*TCHUNK]; [it,p,f,t]=a[it*128+t,32p+f]
        a = a.reshape(NT, TCHUNK, 128, F).transpose(0, 2, 3, 1)
        return np.ascontiguousarray(a).reshape(NT, 128, F * TCHUNK)

    wt = np.ascontiguousarray(w.T)     # [H, E]
    wth = wt.astype(np.float16)
    wtl = (wt - wth.astype(np.float32)).astype(np.float16)

    nbias = np.ascontiguousarray(np.broadcast_to(-b, (128, E)))
    iota = np.ascontiguousarray(
        np.broadcast_to(np.arange(E, dtype=np.float32), (128, E)))

    in_maps = []
    for c in range(N_CORES):
        sl = slice(c * TS, (c + 1) * TS)
        in_maps.append({
            "xth": pretile(xh[sl]),
            "xtl": pretile(xl[sl]),
            "wth": wth,
            "wtl": wtl,
            "nbias": nbias,
            "iota": iota,
        })
    return in_maps


def unpack(out_cores):
    """list of [128, NT*16] u32 -> (inds int32 [T, 8], sel float32 [T, 8])."""
    inds = np.empty((T, K), dtype=np.int32)
    sel = np.empty((T, K), dtype=np.float32)
    for c, o in enumerate(out_cores):
        o = o.reshape(128, NT, 2 * K).transpose(1, 0, 2)  # [it, p, 16]
        o = np.ascontiguousarray(o).reshape(TS, 2 * K)
        inds[c * TS:(c + 1) * TS] = o[:, :K].astype(np.int32)
        sel[c * TS:(c + 1) * TS] = o[:, K:].view(np.float32)
    return inds, sel


_NC_CACHE = {}


def _get_nc():
    if "nc" not in _NC_CACHE:
        _NC_CACHE["nc"] = build_nc()
    return _NC_CACHE["nc"]


def kernel(x, weight, e_score_correction_bias, _trace=False):
    in_maps = host_prep(x, weight, e_score_correction_bias)
    nc = _get_nc()
    res = bass_utils.run_bass_kernel_spmd(
        nc, in_maps, list(range(N_CORES)), trace=_trace)
    inds, sel = unpack([res.results[c]["out"] for c in range(N_CORES)])
    if _trace:
        kernel.last_results = res
    return inds, sel


# revision 23
# speedup vs baseline: 1.0050x; 1.0050x over previous
"""MoE gate kernel for Trainium2 (8 NeuronCores, SPMD data-parallel over tokens).

reference:
    scores = sigmoid(x @ W.T)            # [T, E] fp32
    biased = scores + bias
    inds   = top_k(-biased, 8).indices   # 8 smallest biased, order ascending biased
    sel    = scores[inds] / sum * 2.5

Device strategy (per core, 2048 tokens):
  - x and W.T are split on the host into fp16 hi + fp16 lo residual pairs
    (22-bit mantissa coverage -> logits match the fp32 reference to ~1e-6;
    lo residuals may be fp16 subnormals, which the PE multiplies exactly).
    x is additionally pre-tiled on the host into the exact per-partition
    SBUF layout (h-block on partitions) so every DMA descriptor moves
    contiguous KBs instead of 128B strips.
  - logits = xh@wh + xh@wl + xl@wh  (3 fp16 matmuls, one PSUM accumulation)
    x is the stationary PE operand so PSUM comes out as [tokens, experts]:
    no transposes anywhere on device.
  - ACT sigmoid, DVE max/max_index for top-8 (matches jax tie-breaking),
    iota-equality scalar_tensor_tensor with accum_out to gather the selected
    original scores, reciprocal-normalize, scale by 2.5.
  - Output: one [128, NT*16] u32 buffer; per token 8 idx words + 8 fp32-bit
    sel words; host un-permutes.
"""

import sys

sys.path.insert(0, "/opt/trn_rl_repo")

import numpy as np

import concourse.bacc as bacc
import concourse.mybir as mybir
import concourse.tile as tile
from concourse import bass_utils

T, H, E, K = 16384, 4096, 256, 8
N_CORES = 8
TS = T // N_CORES          # tokens per core
TCHUNK = 128               # tokens per PE tile (PSUM partition dim)
NT = TS // TCHUNK          # token tiles per core
F = H // 128               # h-slices per partition block
ROUTED_SCALING = 2.5

f32 = mybir.dt.float32
f16 = mybir.dt.float16
u32 = mybir.dt.uint32
Alu = mybir.AluOpType
Act = mybir.ActivationFunctionType


def build_nc(nt=NT):
    """Build the SPMD Bass program for one core handling nt*TCHUNK tokens."""
    nc = bacc.Bacc("TRN2", target_bir_lowering=False, debug=False,
                   num_devices=N_CORES)

    # x pre-tiled on host: [it, p, f*TCHUNK + t] = x[it*TCHUNK + t, 32p + f]
    xth_d = nc.dram_tensor("xth", [nt, 128, F * TCHUNK], f16,
                           kind="ExternalInput")
    xtl_d = nc.dram_tensor("xtl", [nt, 128, F * TCHUNK], f16,
                           kind="ExternalInput")
    wth_d = nc.dram_tensor("wth", [H, E], f16, kind="ExternalInput")
    wtl_d = nc.dram_tensor("wtl", [H, E], f16, kind="ExternalInput")
    nbias_d = nc.dram_tensor("nbias", [128, E], f32, kind="ExternalInput")
    iota_d = nc.dram_tensor("iota", [128, E], f32, kind="ExternalInput")
    out_d = nc.dram_tensor("out", [128, nt * 2 * K], u32, kind="ExternalOutput")

    with tile.TileContext(nc) as tc:
        with (
            tc.tile_pool(name="const", bufs=1) as cpool,
            tc.tile_pool(name="xp", bufs=4) as xpool,
            tc.tile_pool(name="sc", bufs=4) as spool,
            tc.tile_pool(name="sm", bufs=4) as smpool,
            tc.tile_pool(name="ps", bufs=4, space="PSUM") as ppool,
        ):
            # weights + consts on the ACT HWDGE queue, x tiles on the SP
            # queue, so the startup loads run in parallel. Each weight chunk
            # is its own tile so the f=0 matmuls depend only on chunk 0.
            FC = F // 4
            wth_src = wth_d.ap().rearrange("(p f) e -> p f e", f=F)
            wtl_src = wtl_d.ap().rearrange("(p f) e -> p f e", f=F)
            # all wth chunks before wtl: the hh matmul phase only needs wth,
            # so the cross-term weights stream while hh matmuls already run.
            wth_c, wtl_c = [], []
            for c in range(4):
                fs = slice(c * FC, (c + 1) * FC)
                th = cpool.tile([128, FC, E], f16, tag=f"wth{c}")
                nc.scalar.dma_start(th[:], wth_src[:, fs, :])
                wth_c.append(th)
            for c in range(4):
                fs = slice(c * FC, (c + 1) * FC)
                tl = cpool.tile([128, FC, E], f16, tag=f"wtl{c}")
                nc.scalar.dma_start(tl[:], wtl_src[:, fs, :])
                wtl_c.append(tl)
            nb = cpool.tile([128, E], f32, tag="nb")
            nc.scalar.dma_start(nb[:], nbias_d.ap())
            io = cpool.tile([128, E], f32, tag="io")
            nc.scalar.dma_start(io[:], iota_d.ap())
            scrv = cpool.tile([128, E], f32, tag="scrv")
            obuf = cpool.tile([128, nt * 2 * K], u32, tag="obuf")

            FH = F // 2
            for it in range(nt):
                xh_src = xth_d.ap()[it].rearrange("p (f t) -> p f t", f=F)
                xl_src = xtl_d.ap()[it].rearrange("p (f t) -> p f t", f=F)
                xh_h, xl_h = [], []
                for c in range(2):
                    fs = slice(c * FH, (c + 1) * FH)
                    th = xpool.tile([128, FH, TCHUNK], f16, tag=f"xh{c}")
                    nc.sync.dma_start(th[:], xh_src[:, fs, :])
                    xh_h.append(th)
                for c in range(2):
                    fs = slice(c * FH, (c + 1) * FH)
                    tl = xpool.tile([128, FH, TCHUNK], f16, tag=f"xl{c}")
                    nc.sync.dma_start(tl[:], xl_src[:, fs, :])
                    xl_h.append(tl)

                # the lo parts are raw fp16 residuals (subnormals included —
                # the PE handles them exactly), so all three terms accumulate
                # into a single PSUM bank.
                acc = ppool.tile([128, E], f32, tag="acc")
                for f in range(F):
                    nc.tensor.matmul(acc[:], xh_h[f // FH][:, f % FH, :],
                                     wth_c[f // FC][:, f % FC, :],
                                     start=(f == 0), stop=False)
                for f in range(F):
                    nc.tensor.matmul(acc[:], xh_h[f // FH][:, f % FH, :],
                                     wtl_c[f // FC][:, f % FC, :],
                                     start=False, stop=False)
                for f in range(F):
                    nc.tensor.matmul(acc[:], xl_h[f // FH][:, f % FH, :],
                                     wth_c[f // FC][:, f % FC, :],
                                     start=False, stop=(f == F - 1))

                scores = spool.tile([128, E], f32, tag="scores")
                nc.scalar.activation(scores[:], acc[:], Act.Sigmoid)

                negb = spool.tile([128, E], f32, tag="negb")
                nc.vector.tensor_tensor(negb[:], nb[:], scores[:], Alu.subtract)
                m8 = smpool.tile([128, K], f32, tag="m8")
                idx = obuf[:, it * 2 * K: it * 2 * K + K]
                nc.vector.max(m8[:], negb[:])
                nc.vector.max_index(idx, m8[:], negb[:])
                idxf = smpool.tile([128, K], f32, tag="idxf")
                nc.vector.tensor_copy(idxf[:], idx)

                gath = smpool.tile([128, K], f32, tag="gath")
                for j in range(K):
                    nc.vector.scalar_tensor_tensor(
                        scrv[:], io[:], idxf[:, j:j + 1], scores[:],
                        Alu.is_equal, Alu.mult,
                        accum_out=gath[:, j:j + 1])

                ssum = smpool.tile([128, 1], f32, tag="ssum")
                nc.vector.tensor_reduce(ssum[:], gath[:],
                                        mybir.AxisListType.X, Alu.add)
                rec = smpool.tile([128, 1], f32, tag="rec")
                nc.vector.reciprocal(rec[:], ssum[:])

                nc.vector.tensor_scalar(
                    obuf[:, it * 2 * K + K: (it + 1) * 2 * K].bitcast(f32),
                    gath[:], rec[:], ROUTED_SCALING, Alu.mult, Alu.mult)

            nc.sync.dma_start(out_d.ap(), obuf[:])

    nc.compile()
    return nc


def host_prep(x, weight, e_score_correction_bias):
    """Split inputs into fp16 hi/lo pairs, pre-tile x, build per-core maps."""
    x = np.asarray(x, dtype=np.float32)
    w = np.asarray(weight, dtype=np.float32)
    b = np.asarray(e_score_correction_bias, dtype=np.float32)

    xh = x.astype(np.float16)
    xl = (x - xh.astype(np.float32)).astype(np.float16)

    def pretile(a):  # [TS, H] -> [NT, 128, F*TCHUNK]; [it,p,f,t]=a[it*128+t,32p+f]
        a = a.reshape(NT, TCHUNK, 128, F).transpose(0, 2, 3, 1)
        return np.ascontiguousarray(a).reshape(NT, 128, F * TCHUNK)

    wt = np.ascontiguousarray(w.T)     # [H, E]
    wth = wt.astype(np.float16)
    wtl = (wt - wth.astype(np.float32)).astype(np.float16)

    nbias = np.ascontiguousarray(np.broadcast_to(-b, (128, E)))
    iota = np.ascontiguousarray(
        np.broadcast_to(np.arange(E, dtype=np.float32), (128, E)))

    in_maps = []
    for c in range(N_CORES):
        sl = slice(c * TS, (c + 1) * TS)
        in_maps.append({
            "xth": pretile(xh[sl]),
            "xtl": pretile(xl[sl]),
            "wth": wth,
            "wtl": wtl,
            "nbias": nbias,
            "iota": iota,
        })
    return in_maps


def unpack(out_cores):
    """list of [128, NT*16] u32 -> (inds int32 [T, 8], sel float32 [T, 8])."""
    inds = np.empty((T, K), dtype=np.int32)
    sel = np.empty((T, K), dtype=np.float32)
    for c, o in enumerate(out_cores):
        o = o.reshape(128, NT, 2 * K).transpose(1, 0, 2)  # [it, p, 16]
        o = np.ascontiguousarray(o).reshape(TS, 2 * K)
        inds[c * TS:(c + 1) * TS] = o[:, :K].astype(np.int32)
        sel[c * TS:(c + 1) * TS] = o[:, K:].view(np.float32)
    return inds, sel


_NC_CACHE = {}


def _get_nc():
    if "nc" not in _NC_CACHE:
        _NC_CACHE["nc"] = build_nc()
    return _NC_CACHE["nc"]


def kernel(x, weight, e_score_correction_bias, _trace=False):
    in_maps = host_prep(x, weight, e_score_correction_bias)
    nc = _get_nc()
    res = bass_utils.run_bass_kernel_spmd(
        nc, in_maps, list(range(N_CORES)), trace=_trace)
    inds, sel = unpack([res.results[c]["out"] for c in range(N_CORES)])
    if _trace:
        kernel.last_results = res
    return inds, sel


# revision 24
# speedup vs baseline: 1.0320x; 1.0269x over previous
"""MoE gate kernel for Trainium2 (8 NeuronCores, SPMD data-parallel over tokens).

reference:
    scores = sigmoid(x @ W.T)            # [T, E] fp32
    biased = scores + bias
    inds   = top_k(-biased, 8).indices   # 8 smallest biased, order ascending biased
    sel    = scores[inds] / sum * 2.5

Device strategy (per core, 2048 tokens):
  - x and W.T are split on the host into fp16 hi + fp16 lo residual pairs
    (22-bit mantissa coverage -> logits match the fp32 reference to ~1e-6;
    lo residuals may be fp16 subnormals, which the PE multiplies exactly).
    x is additionally pre-tiled on the host into the exact per-partition
    SBUF layout (h-block on partitions) so every DMA descriptor moves
    contiguous KBs instead of 128B strips.
  - logits = xh@wh + xh@wl + xl@wh  (3 fp16 matmuls, one PSUM accumulation)
    x is the stationary PE operand so PSUM comes out as [tokens, experts]:
    no transposes anywhere on device.
  - ACT sigmoid, DVE max/max_index for top-8 (matches jax tie-breaking),
    iota-equality scalar_tensor_tensor with accum_out to gather the selected
    original scores, reciprocal-normalize, scale by 2.5.
  - Output: one [128, NT*16] u32 buffer; per token 8 idx words + 8 fp32-bit
    sel words; host un-permutes.
"""

import sys

sys.path.insert(0, "/opt/trn_rl_repo")

import numpy as np

import concourse.bacc as bacc
import concourse.mybir as mybir
import concourse.tile as tile
from concourse import bass_utils

T, H, E, K = 16384, 4096, 256, 8
N_CORES = 8
TS = T // N_CORES          # tokens per core
TCHUNK = 128               # tokens per PE tile (PSUM partition dim)
NT = TS // TCHUNK          # token tiles per core
F = H // 128               # h-slices per partition block
ROUTED_SCALING = 2.5

f32 = mybir.dt.float32
f16 = mybir.dt.float16
u32 = mybir.dt.uint32
Alu = mybir.AluOpType
Act = mybir.ActivationFunctionType


def build_nc(nt=NT):
    """Build the SPMD Bass program for one core handling nt*TCHUNK tokens."""
    nc = bacc.Bacc("TRN2", target_bir_lowering=False, debug=False,
                   num_devices=N_CORES)

    # x pre-tiled on host: [it, p, f*TCHUNK + t] = x[it*TCHUNK + t, 32p + f]
    xth_d = nc.dram_tensor("xth", [nt, 128, F * TCHUNK], f16,
                           kind="ExternalInput")
    xtl_d = nc.dram_tensor("xtl", [nt, 128, F * TCHUNK], f16,
                           kind="ExternalInput")
    wth_d = nc.dram_tensor("wth", [H, E], f16, kind="ExternalInput")
    wtl_d = nc.dram_tensor("wtl", [H, E], f16, kind="ExternalInput")
    nbias_d = nc.dram_tensor("nbias", [128, E], f32, kind="ExternalInput")
    iota_d = nc.dram_tensor("iota", [128, E], f32, kind="ExternalInput")
    out_d = nc.dram_tensor("out", [128, nt * 2 * K], u32, kind="ExternalOutput")

    with tile.TileContext(nc) as tc:
        with (
            tc.tile_pool(name="const", bufs=1) as cpool,
            tc.tile_pool(name="xp", bufs=4) as xpool,
            tc.tile_pool(name="sc", bufs=4) as spool,
            tc.tile_pool(name="sm", bufs=4) as smpool,
            tc.tile_pool(name="ps", bufs=8, space="PSUM") as ppool,
        ):
            # weights + consts on the ACT HWDGE queue, x tiles on the SP
            # queue, so the startup loads run in parallel. Each weight chunk
            # is its own tile so the f=0 matmuls depend only on chunk 0.
            FC = F // 8
            wth_src = wth_d.ap().rearrange("(p f) e -> p f e", f=F)
            wtl_src = wtl_d.ap().rearrange("(p f) e -> p f e", f=F)
            # all wth chunks before wtl: the hh matmul phase only needs wth,
            # so the cross-term weights stream while hh matmuls already run.
            wth_c, wtl_c = [], []
            for c in range(8):
                fs = slice(c * FC, (c + 1) * FC)
                th = cpool.tile([128, FC, E], f16, tag=f"wth{c}")
                nc.scalar.dma_start(th[:], wth_src[:, fs, :])
                wth_c.append(th)
            for c in range(8):
                fs = slice(c * FC, (c + 1) * FC)
                tl = cpool.tile([128, FC, E], f16, tag=f"wtl{c}")
                nc.scalar.dma_start(tl[:], wtl_src[:, fs, :])
                wtl_c.append(tl)
            nb = cpool.tile([128, E], f32, tag="nb")
            nc.scalar.dma_start(nb[:], nbias_d.ap())
            io = cpool.tile([128, E], f32, tag="io")
            nc.scalar.dma_start(io[:], iota_d.ap())
            scrv = cpool.tile([128, E], f32, tag="scrv")
            obuf = cpool.tile([128, nt * 2 * K], u32, tag="obuf")

            FH = F // 4
            for it in range(nt):
                xh_src = xth_d.ap()[it].rearrange("p (f t) -> p f t", f=F)
                xl_src = xtl_d.ap()[it].rearrange("p (f t) -> p f t", f=F)
                xh_h, xl_h = [], []
                for c in range(4):
                    fs = slice(c * FH, (c + 1) * FH)
                    th = xpool.tile([128, FH, TCHUNK], f16, tag=f"xh{c}")
                    nc.sync.dma_start(th[:], xh_src[:, fs, :])
                    xh_h.append(th)
                for c in range(4):
                    fs = slice(c * FH, (c + 1) * FH)
                    tl = xpool.tile([128, FH, TCHUNK], f16, tag=f"xl{c}")
                    nc.sync.dma_start(tl[:], xl_src[:, fs, :])
                    xl_h.append(tl)

                # the lo parts are raw fp16 residuals (subnormals included —
                # the PE handles them exactly), so all three terms accumulate
                # into a single PSUM bank.
                acc = ppool.tile([128, E], f32, tag="acc")
                for f in range(F):
                    nc.tensor.matmul(acc[:], xh_h[f // FH][:, f % FH, :],
                                     wth_c[f // FC][:, f % FC, :],
                                     start=(f == 0), stop=False)
                for f in range(F):
                    nc.tensor.matmul(acc[:], xh_h[f // FH][:, f % FH, :],
                                     wtl_c[f // FC][:, f % FC, :],
                                     start=False, stop=False)
                for f in range(F):
                    nc.tensor.matmul(acc[:], xl_h[f // FH][:, f % FH, :],
                                     wth_c[f // FC][:, f % FC, :],
                                     start=False, stop=(f == F - 1))

                scores = spool.tile([128, E], f32, tag="scores")
                nc.scalar.activation(scores[:], acc[:], Act.Sigmoid)

                negb = spool.tile([128, E], f32, tag="negb")
                nc.vector.tensor_tensor(negb[:], nb[:], scores[:], Alu.subtract)
                m8 = smpool.tile([128, K], f32, tag="m8")
                idx = obuf[:, it * 2 * K: it * 2 * K + K]
                nc.vector.max(m8[:], negb[:])
                nc.vector.max_index(idx, m8[:], negb[:])
                idxf = smpool.tile([128, K], f32, tag="idxf")
                nc.vector.tensor_copy(idxf[:], idx)

                gath = smpool.tile([128, K], f32, tag="gath")
                for j in range(K):
                    nc.vector.scalar_tensor_tensor(
                        scrv[:], io[:], idxf[:, j:j + 1], scores[:],
                        Alu.is_equal, Alu.mult,
                        accum_out=gath[:, j:j + 1])

                ssum = smpool.tile([128, 1], f32, tag="ssum")
                nc.vector.tensor_reduce(ssum[:], gath[:],
                                        mybir.AxisListType.X, Alu.add)
                rec = smpool.tile([128, 1], f32, tag="rec")
                nc.vector.reciprocal(rec[:], ssum[:])

                nc.vector.tensor_scalar(
                    obuf[:, it * 2 * K + K: (it + 1) * 2 * K].bitcast(f32),
                    gath[:], rec[:], ROUTED_SCALING, Alu.mult, Alu.mult)

            nc.sync.dma_start(out_d.ap(), obuf[:])

    nc.compile()
    return nc


def host_prep(x, weight, e_score_correction_bias):
    """Split inputs into fp16 hi/lo pairs, pre-tile x, build per-core maps."""
    x = np.asarray(x, dtype=np.float32)
    w = np.asarray(weight, dtype=np.float32)
    b = np.asarray(e_score_correction_bias, dtype=np.float32)

    xh = x.astype(np.float16)
    xl = (x - xh.astype(np.float32)).astype(np.float16)

    def pretile(a):  # [TS, H] -> [NT, 128, F*TCHUNK]; [it,p,f,t]=a[it*128+t,32p+f]
        a = a.reshape(NT, TCHUNK, 128, F).transpose(0, 2, 3, 1)
        return np.ascontiguousarray(a).reshape(NT, 128, F * TCHUNK)

    wt = np.ascontiguousarray(w.T)     # [H, E]
    wth = wt.astype(np.float16)
    wtl = (wt - wth.astype(np.float32)).astype(np.float16)

    nbias = np.ascontiguousarray(np.broadcast_to(-b, (128, E)))
    iota = np.ascontiguousarray(
        np.broadcast_to(np.arange(E, dtype=np.float32), (128, E)))

    in_maps = []
    for c in range(N_CORES):
        sl = slice(c * TS, (c + 1) * TS)
        in_maps.append({
            "xth": pretile(xh[sl]),
            "xtl": pretile(xl[sl]),
            "wth": wth,
            "wtl": wtl,
            "nbias": nbias,
            "iota": iota,
        })
    return in_maps


def unpack(out_cores):
    """list of [128, NT*16] u32 -> (inds int32 [T, 8], sel float32 [T, 8])."""
    inds = np.empty((T, K), dtype=np.int32)
    sel = np.empty((T, K), dtype=np.float32)
    for c, o in enumerate(out_cores):
        o = o.reshape(128, NT, 2 * K).transpose(1, 0, 2)  # [it, p, 16]
        o = np.ascontiguousarray(o).reshape(TS, 2 * K)
        inds[c * TS:(c + 1) * TS] = o[:, :K].astype(np.int32)
        sel[c * TS:(c + 1) * TS] = o[:, K:].view(np.float32)
    return inds, sel


_NC_CACHE = {}


def _get_nc():
    if "nc" not in _NC_CACHE:
        _NC_CACHE["nc"] = build_nc()
    return _NC_CACHE["nc"]


def kernel(x, weight, e_score_correction_bias, _trace=False):
    in_maps = host_prep(x, weight, e_score_correction_bias)
    nc = _get_nc()
    res = bass_utils.run_bass_kernel_spmd(
        nc, in_maps, list(range(N_CORES)), trace=_trace)
    inds, sel = unpack([res.results[c]["out"] for c in range(N_CORES)])
    if _trace:
        kernel.last_results = res
    return inds, sel
